# revision 1
# baseline (speedup 1.0000x reference)
"""Trainium2 Bass kernel for nn_DeepModel_multi_12945031430869.

Computes, for heads h in 0..31:
    y[:, h] = relu(x @ W1[h] + b1[h]) @ W2[h] + b2[h]
    out[:, h*513:(h+1)*513] = [x, y[:, h]]          # [4096, 16416]

Sharding: head-parallel across 8 NeuronCores (4 heads per core). Each core
produces its own [4096, 4*513] column block; the host concatenates them.

Per-core device program:
  - First GEMM on the PE array in fp32r (full rate for N>=512 moving dim):
    psum[128 rows, 512 dh] = sum_k xT[k, rt].T @ W1[k, dh]  (2048 matmuls)
  - Epilogue on the Vector engine using relu(v+b) = max(v,-b) + b:
      t    = max(psum, -b1)                       (tensor_tensor)
      p_t  = sum_dh t * W2                        (scalar_tensor_tensor accum)
    then the 4 dh-tile partials p_t are combined on the Scalar engine via
    activation(Identity, bias=b2eff/4, accum_out=y): y = sum_t (p_t + b2eff/4)
    where b2eff = b2 + sum_f W2[h,f]*b1[h,f] is folded on the host.
  - Output block [128, 513] assembled in SBUF (x copied in by DMA, y written
    by the last reduce directly into column 512), one DMA per block out.
"""

import numpy as np

N = 4096
D_IN = 512
D_H = 2048
USED = 32
NCORES = 8
HPC = USED // NCORES  # heads per core = 4
KT = D_IN // 128      # k tiles = 4
TT = D_H // 512       # dh tiles of 512 = 4
RT = N // 128         # row tiles = 32

_PROG = None


def _build_program():
    import concourse.tile as tile
    import concourse.mybir as mybir
    from concourse import bacc

    f32 = mybir.dt.float32
    f32r = mybir.dt.float32r
    bf16 = mybir.dt.bfloat16

    nc = bacc.Bacc("TRN2", target_bir_lowering=False, debug=False)

    xT_d = nc.dram_tensor("xT", [KT, 128, N], f32r, kind="ExternalInput").ap()
    x_d = nc.dram_tensor("x", [N, D_IN], f32, kind="ExternalInput").ap()
    w1_d = nc.dram_tensor("w1", [HPC, TT, 128, KT * 512], f32r, kind="ExternalInput").ap()
    nb1_d = nc.dram_tensor("negb1", [128, HPC * D_H], bf16, kind="ExternalInput").ap()
    w2_d = nc.dram_tensor("w2", [128, HPC * D_H], f32, kind="ExternalInput").ap()
    b2_d = nc.dram_tensor("b2r", [128, HPC], f32, kind="ExternalInput").ap()
    out_d = nc.dram_tensor("out", [N, HPC * 513], f32, kind="ExternalOutput").ap()

    with tile.TileContext(nc) as tc:
        with tc.tile_pool(name="xt", bufs=1) as xtp, \
             tc.tile_pool(name="cst", bufs=1) as cst, \
             tc.tile_pool(name="w1p", bufs=6) as w1p, \
             tc.tile_pool(name="ps", bufs=4, space="PSUM") as pp, \
             tc.tile_pool(name="tmax", bufs=3) as tmp_, \
             tc.tile_pool(name="scr", bufs=3) as scr, \
             tc.tile_pool(name="yp", bufs=4) as yp, \
             tc.tile_pool(name="j4", bufs=4) as j4p, \
             tc.tile_pool(name="ob", bufs=6) as obp:

            xts = []
            for k in range(KT):
                t = xtp.tile([128, N], f32r, tag=f"x{k}")
                nc.sync.dma_start(t[:], xT_d[k])
                xts.append(t)
            nb1 = cst.tile([128, HPC * D_H], bf16, tag="nb1")
            nc.sync.dma_start(nb1[:], nb1_d[:])
            w2 = cst.tile([128, HPC * D_H], f32, tag="w2")
            nc.sync.dma_start(w2[:], w2_d[:])
            b2r = cst.tile([128, HPC], f32, tag="b2r")
            nc.sync.dma_start(b2r[:], b2_d[:])

            mx = mybir.AluOpType.max
            mult = mybir.AluOpType.mult
            ident = mybir.ActivationFunctionType.Identity

            for h in range(HPC):
                blks = []
                for t in range(TT):
                    b = w1p.tile([128, KT * 512], f32r, tag="w1")
                    nc.sync.dma_start(b[:], w1_d[h, t])
                    blks.append(b)
                for rt in range(RT):
                    rs = rt * 128
                    ob = obp.tile([128, 513], f32, tag="ob")
                    nc.sync.dma_start(ob[:, 0:512], x_d[rs:rs + 128, :])
                    yp4 = yp.tile([128, TT], f32, tag="y")
                    for t in range(TT):
                        ps = pp.tile([128, 512], f32, tag="ps")
                        for k in range(KT):
                            nc.tensor.matmul(
                                ps[:],
                                lhsT=xts[k][:, rs:rs + 128],
                                rhs=blks[t][:, k * 512:(k + 1) * 512],
                                start=(k == 0),
                                stop=(k == KT - 1),
                            )
                        c0 = h * D_H + t * 512
                        tt_ = tmp_.tile([128, 512], f32, tag="t")
                        nc.vector.tensor_tensor(tt_[:], ps[:], nb1[:, c0:c0 + 512], op=mx)
                        sc = scr.tile([128, 512], f32, tag="s")
                        nc.vector.scalar_tensor_tensor(
                            out=sc[:],
                            in0=tt_[:],
                            scalar=1.0,
                            in1=w2[:, c0:c0 + 512],
                            op0=mult,
                            op1=mult,
                            accum_out=yp4[:, t:t + 1],
                        )
                    j4 = j4p.tile([128, TT], f32, tag="j")
                    nc.scalar.activation(
                        j4[:], yp4[:], ident,
                        bias=b2r[:, h:h + 1], scale=1.0,
                        accum_out=ob[:, 512:513],
                    )
                    nc.sync.dma_start(
                        out_d[rs:rs + 128, h * 513:(h + 1) * 513], ob[:]
                    )

    nc.compile()
    return nc


def _get_program():
    global _PROG
    if _PROG is None:
        _PROG = _build_program()
    return _PROG


def kernel(x, W1, b1, W2, b2):
    import ml_dtypes
    from concourse.bass_utils import run_bass_kernel_spmd

    x = np.asarray(x, dtype=np.float32)
    W1 = np.asarray(W1, dtype=np.float32)
    b1 = np.asarray(b1, dtype=np.float32)
    W2 = np.asarray(W2, dtype=np.float32)
    b2 = np.asarray(b2, dtype=np.float32)

    nc = _get_program()

    xT4 = np.ascontiguousarray(x.T).reshape(KT, 128, N)

    in_maps = []
    for c in range(NCORES):
        hs = slice(HPC * c, HPC * (c + 1))
        w1c = W1[hs]  # [HPC, 512, 2048]
        w1r = np.ascontiguousarray(
            w1c.reshape(HPC, KT, 128, TT, 512).transpose(0, 3, 2, 1, 4)
        ).reshape(HPC, TT, 128, KT * 512)
        nb1 = np.broadcast_to(
            (-b1[hs]).reshape(1, HPC * D_H).astype(ml_dtypes.bfloat16),
            (128, HPC * D_H),
        )
        w2r = np.broadcast_to(W2[hs].reshape(1, HPC * D_H), (128, HPC * D_H))
        b2eff = (
            b2[hs].astype(np.float64)
            + np.einsum("hf,hf->h", W2[hs].astype(np.float64), b1[hs].astype(np.float64))
        ) / TT  # bias is applied to each of the TT partials before the accum-sum
        b2r = np.broadcast_to(b2eff.astype(np.float32).reshape(1, HPC), (128, HPC))
        in_maps.append({
            "xT": xT4,
            "x": x,
            "w1": w1r,
            "negb1": np.ascontiguousarray(nb1),
            "w2": np.ascontiguousarray(w2r),
            "b2r": np.ascontiguousarray(b2r),
        })

    import os
    trace = os.environ.get("BASS_KERNEL_TRACE") == "1"
    if trace:
        import sys
        sys.path.insert(0, "/tmp")
        try:
            import axon_shim
            axon_shim.install()
        except Exception:
            trace = False
    res = run_bass_kernel_spmd(nc, in_maps, list(range(NCORES)), trace=trace)
    kernel.last_result = res

    return np.concatenate([res.results[c]["out"] for c in range(NCORES)], axis=1)



# revision 3
# speedup vs baseline: 1.0771x; 1.0771x over previous
"""Trainium2 Bass kernel for nn_DeepModel_multi_12945031430869.

Computes, for heads h in 0..31:
    y[:, h] = relu(x @ W1[h] + b1[h]) @ W2[h] + b2[h]
    out[:, h*513:(h+1)*513] = [x, y[:, h]]          # [4096, 16416]

Sharding: head-parallel across 8 NeuronCores (4 heads per core). Each core
produces its own [4096, 4*513] column block; the host concatenates them.

Per-core device program (v2):
  - GEMM1 on the PE array in bf16 (fp32r measured ~395-428 ns per N=512
    matmul on HW vs ~216 ns bf16). Per (head, row-tile): 16 matmuls
    (k-outer so each stationary x-block is loaded once and streams 4 rhs
    tiles) accumulate a [128, 2048] PSUM supertile (4 banks); PSUM pool
    bufs=2 uses all 8 banks and double-buffers against the epilogue.
  - Epilogue: |w2| is folded into W1 column-wise on the host and columns
    are sorted by descending w2, so GEMM2 collapses into sign-partitioned
    sums of max-terms that single vector-engine scalar_tensor_tensor ops
    produce directly via accum_out:
      region A = cols [0, MLO)      all w2 > 0, folded:
         (ps * +1) max (-|w2|b1)  -> accum = sum C_f - sum |w2|b1
      region B = cols [MHI, 2048)  all w2 < 0, folded:
         (ps * -1) min (+|w2|b1)  -> accum = sum C_f + sum |w2|b1
      region M = cols [MLO, MHI)   mixed signs near the boundary, raw:
         m = max(ps, -b1)  (tensor_tensor)
         (m * 1) * w2     -> accum = sum C_f - sum w2 b1
    where C_f = w2_f relu(z_f + b1_f). The constant residues sum to
    b2eff = b2 + sum_f w2_f b1_f, applied on the Scalar engine via
    activation(Identity, bias=b2eff/3, accum_out=y) over the 3 partials,
    writing y directly into the output block's last column.
  - Output block [128, 513] assembled in SBUF (x copied in by DMA), one
    DMA per block out.

The region boundaries (MLO, MHI) are data-independent for any input whose
per-head positive counts fall in [MLO, MHI] (true at 2.6 sigma margin for
gaussian w2; asserted per call, widened + rebuilt if violated).
"""

import numpy as np

N = 4096
D_IN = 512
D_H = 2048
USED = 32
NCORES = 8
HPC = USED // NCORES  # heads per core = 4
KT = D_IN // 128      # k tiles = 4
TT = D_H // 512       # dh tiles of 512 = 4
RT = N // 128         # row tiles = 32

_PROGS = {}


def _build(mlo, mhi):
    import concourse.tile as tile
    import concourse.mybir as mybir
    from concourse import bacc

    f32 = mybir.dt.float32
    bf16 = mybir.dt.bfloat16
    mw = mhi - mlo

    nc = bacc.Bacc("TRN2", target_bir_lowering=False, debug=False)

    xT_d = nc.dram_tensor("xT", [KT, 128, N], bf16, kind="ExternalInput").ap()
    x_d = nc.dram_tensor("x", [N, D_IN], f32, kind="ExternalInput").ap()
    w1_d = nc.dram_tensor("w1", [HPC, 128, KT * D_H], bf16, kind="ExternalInput").ap()
    sb1_d = nc.dram_tensor("sb1", [128, HPC * D_H], f32, kind="ExternalInput").ap()
    w2m_d = nc.dram_tensor("w2m", [128, HPC * mw], f32, kind="ExternalInput").ap()
    b2_d = nc.dram_tensor("b2r", [128, HPC], f32, kind="ExternalInput").ap()
    out_d = nc.dram_tensor("out", [N, HPC * 513], f32, kind="ExternalOutput").ap()

    with tile.TileContext(nc) as tc:
        with tc.tile_pool(name="xt", bufs=1) as xtp, \
             tc.tile_pool(name="cst", bufs=1) as cst, \
             tc.tile_pool(name="w1p", bufs=2) as w1p, \
             tc.tile_pool(name="ps", bufs=2, space="PSUM") as pp, \
             tc.tile_pool(name="scr", bufs=3) as scr, \
             tc.tile_pool(name="mm", bufs=3) as mmp, \
             tc.tile_pool(name="acc", bufs=4) as accp, \
             tc.tile_pool(name="j3", bufs=4) as j3p, \
             tc.tile_pool(name="ob", bufs=6) as obp:

            xts = []
            for k in range(KT):
                t = xtp.tile([128, N], bf16, tag=f"x{k}")
                nc.sync.dma_start(t[:], xT_d[k])
                xts.append(t)
            sb1 = cst.tile([128, HPC * D_H], f32, tag="sb1")
            nc.sync.dma_start(sb1[:], sb1_d[:])
            w2m = cst.tile([128, HPC * mw], f32, tag="w2m")
            nc.sync.dma_start(w2m[:], w2m_d[:])
            b2r = cst.tile([128, HPC], f32, tag="b2r")
            nc.sync.dma_start(b2r[:], b2_d[:])

            mx = mybir.AluOpType.max
            mn = mybir.AluOpType.min
            mult = mybir.AluOpType.mult
            ident = mybir.ActivationFunctionType.Identity

            for h in range(HPC):
                w1t = w1p.tile([128, KT * D_H], bf16, tag="w1")
                nc.sync.dma_start(w1t[:], w1_d[h])
                for rt in range(RT):
                    rs = rt * 128
                    ob = obp.tile([128, 513], f32, tag="ob")
                    nc.sync.dma_start(ob[:, 0:512], x_d[rs:rs + 128, :])
                    ps = pp.tile([128, D_H], f32, tag="ps")
                    for k in range(KT):
                        for t in range(TT):
                            nc.tensor.matmul(
                                ps[:, t * 512:(t + 1) * 512],
                                lhsT=xts[k][:, rs:rs + 128],
                                rhs=w1t[:, k * D_H + t * 512:k * D_H + (t + 1) * 512],
                                start=(k == 0),
                                stop=(k == KT - 1),
                            )
                    c0 = h * D_H
                    acc = accp.tile([128, 3], f32, tag="acc")
                    sc = scr.tile([128, D_H], bf16, tag="sc")
                    nc.vector.scalar_tensor_tensor(
                        out=sc[:, 0:mlo],
                        in0=ps[:, 0:mlo],
                        scalar=1.0,
                        in1=sb1[:, c0:c0 + mlo],
                        op0=mult,
                        op1=mx,
                        accum_out=acc[:, 0:1],
                    )
                    m = mmp.tile([128, mw], f32, tag="m")
                    nc.vector.tensor_tensor(
                        m[:], ps[:, mlo:mhi], sb1[:, c0 + mlo:c0 + mhi], op=mx
                    )
                    nc.vector.scalar_tensor_tensor(
                        out=sc[:, mlo:mhi],
                        in0=m[:],
                        scalar=1.0,
                        in1=w2m[:, h * mw:(h + 1) * mw],
                        op0=mult,
                        op1=mult,
                        accum_out=acc[:, 1:2],
                    )
                    nc.vector.scalar_tensor_tensor(
                        out=sc[:, mhi:D_H],
                        in0=ps[:, mhi:D_H],
                        scalar=-1.0,
                        in1=sb1[:, c0 + mhi:c0 + D_H],
                        op0=mult,
                        op1=mn,
                        accum_out=acc[:, 2:3],
                    )
                    j3 = j3p.tile([128, 3], f32, tag="j3")
                    nc.scalar.activation(
                        j3[:], acc[:], ident,
                        bias=b2r[:, h:h + 1], scale=1.0,
                        accum_out=ob[:, 512:513],
                    )
                    nc.sync.dma_start(
                        out_d[rs:rs + 128, h * 513:(h + 1) * 513], ob[:]
                    )

    nc.compile()
    return nc


def _get_program(mlo, mhi):
    key = (mlo, mhi)
    if key not in _PROGS:
        _PROGS[key] = _build(mlo, mhi)
    return _PROGS[key]


def kernel(x, W1, b1, W2, b2):
    import ml_dtypes
    from concourse.bass_utils import run_bass_kernel_spmd

    x = np.asarray(x, dtype=np.float32)
    W1 = np.asarray(W1, dtype=np.float32)
    b1 = np.asarray(b1, dtype=np.float32)
    W2 = np.asarray(W2, dtype=np.float32)
    b2 = np.asarray(b2, dtype=np.float32)

    # region boundaries: widen from the default if this input's sign
    # pattern demands it (keeps the program data-independent in the
    # common case)
    P = (W2[:USED] > 0).sum(axis=1)
    mlo, mhi = 960, 1088
    if P.min() < mlo:
        mlo = int(P.min()) // 64 * 64
    if P.max() > mhi:
        mhi = -(-int(P.max()) // 64) * 64
    mw = mhi - mlo

    nc = _get_program(mlo, mhi)

    xT = np.ascontiguousarray(x.T).astype(ml_dtypes.bfloat16).reshape(KT, 128, N)

    in_maps = []
    for c in range(NCORES):
        w1heads = []
        sb1cols = np.empty(HPC * D_H, dtype=np.float32)
        w2mcols = np.empty(HPC * mw, dtype=np.float32)
        b2eff = np.empty(HPC, dtype=np.float32)
        for i in range(HPC):
            h = HPC * c + i
            w2 = W2[h]
            order = np.argsort(-w2, kind="stable")  # descending w2
            w2s = w2[order]
            b1s = b1[h][order]
            assert w2s[mlo - 1] > 0 and w2s[mhi] < 0, "region overflow"
            w1s = W1[h][:, order].copy()  # [512, 2048]
            aw = np.abs(w2s)
            scale = aw.copy()
            scale[mlo:mhi] = 1.0  # M region stays unfolded
            w1s *= scale[None, :]
            sb = np.empty(D_H, dtype=np.float32)
            sb[:mlo] = -aw[:mlo] * b1s[:mlo]
            sb[mlo:mhi] = -b1s[mlo:mhi]
            sb[mhi:] = aw[mhi:] * b1s[mhi:]
            sb1cols[i * D_H:(i + 1) * D_H] = sb
            w2mcols[i * mw:(i + 1) * mw] = w2s[mlo:mhi]
            b2eff[i] = (
                b2[h].astype(np.float64)
                + np.dot(w2.astype(np.float64), b1[h].astype(np.float64))
            ) / 3.0
            # [512, 2048] -> [128 part, KT*D_H] with cols = k*D_H + j
            w1heads.append(
                np.ascontiguousarray(
                    w1s.reshape(KT, 128, D_H).transpose(1, 0, 2)
                ).reshape(128, KT * D_H)
            )
        in_maps.append({
            "xT": xT,
            "x": x,
            "w1": np.ascontiguousarray(
                np.stack(w1heads, axis=0).astype(ml_dtypes.bfloat16)
            ),
            "sb1": np.ascontiguousarray(
                np.broadcast_to(sb1cols.reshape(1, -1), (128, HPC * D_H))
            ),
            "w2m": np.ascontiguousarray(
                np.broadcast_to(w2mcols.reshape(1, -1), (128, HPC * mw))
            ),
            "b2r": np.ascontiguousarray(
                np.broadcast_to(b2eff.reshape(1, -1), (128, HPC))
            ),
        })

    import os
    trace = os.environ.get("BASS_KERNEL_TRACE") == "1"
    if trace:
        import sys
        sys.path.insert(0, "/tmp")
        try:
            import axon_shim
            axon_shim.install()
        except Exception:
            trace = False
    res = run_bass_kernel_spmd(nc, in_maps, list(range(NCORES)), trace=trace)
    kernel.last_result = res

    return np.concatenate([res.results[c]["out"] for c in range(NCORES)], axis=1)
